# revision 8
# baseline (speedup 1.0000x reference)
"""LlamaTernaryMLP (SwiGLU MLP with ternary weights + per-channel scales) on 8 TRN2 cores.

Strategy: pure data-parallel over the 8192 tokens (1024 tokens/core, no
collectives).  Each core runs the full MLP on its token shard with all three
weight matrices streamed from HBM in bf16 (ternary values are exact in bf16;
only the activations lose precision, fp32 accumulation in PSUM).

Layout (host-prepped so every DMA is >=1KB-contiguous per partition):
  activations are kept feature-major on chip ([feature,token]); weights are
  pre-transposed/tiled so each matmul is lhsT=[K=128, M=128] stationary weight
  tile against a moving [K=128, N=512] activation tile.

Per core, per 512-token block:
  gate/up: for each of 86 I-tiles accumulate 32 K-tiles over HIDDEN into two
  PSUM banks, then h = silu(gate*ag) * (up*au) -> bf16 SBUF (86 tiles resident).
  down:    for each of 32 H-tiles accumulate 86 K-tiles over INTER, scale by ad,
  DMA out fp32.
"""

import numpy as np
import ml_dtypes

import concourse.bass as bass
import concourse.mybir as mybir
import concourse.tile as tile
from concourse import bacc
from concourse.bass_utils import run_bass_kernel_spmd

P = 128
B, S, HID, INT = 4, 2048, 4096, 11008
NCORES = 8
BLK = 512
CS = 128  # matmul column-strip width (128 = no column tiling; 32/64 tested slower)
# Weight dtype: ternary {-1,0,+1} is exact in fp8e4m3.  fp8 stationary halves
# the FWL weight-load time (4 values per 32-bit read vs 2 for bf16) and the
# weight DMA traffic; the moving activations stay bf16 (mixed-dtype matmul).
WDT = "f8"  # "f8" | "bf16"


def build_nc(t_loc, hid, inter, blk=BLK, reps=1):
    """Build the per-core Bass program for a t_loc-token shard.

    reps>1 wraps the whole computation in a hardware For_i loop (identical
    compute each iteration) — benchmarking only, so per-iteration time can be
    extracted from wall-clock above the axon RPC floor.
    """
    nblk = t_loc // blk
    kt = hid // P    # K-tiles over hidden (gate/up contraction)
    it = inter // P  # I-tiles (intermediate channels / down contraction)
    ht = hid // P    # output H-tiles
    bf16 = mybir.dt.bfloat16
    f32 = mybir.dt.float32
    wdt = mybir.dt.float8e4 if WDT == "f8" else bf16
    AF = mybir.ActivationFunctionType
    OP = mybir.AluOpType

    nc = bacc.Bacc(
        "TRN2", target_bir_lowering=False, debug=False, num_devices=NCORES
    )
    xp = nc.declare_dram_parameter("xp", [nblk, P, kt, blk], bf16, isOutput=False)
    wgp = nc.declare_dram_parameter("wgp", [it, P, kt, P], wdt, isOutput=False)
    wup = nc.declare_dram_parameter("wup", [it, P, kt, P], wdt, isOutput=False)
    wdp = nc.declare_dram_parameter("wdp", [ht, P, it, P], wdt, isOutput=False)
    ags = nc.declare_dram_parameter("ags", [P, it], f32, isOutput=False)
    aus = nc.declare_dram_parameter("aus", [P, it], f32, isOutput=False)
    ads = nc.declare_dram_parameter("ads", [P, ht], f32, isOutput=False)
    outp = nc.declare_dram_parameter("outp", [nblk, ht, P, blk], f32, isOutput=True)

    half = (it + 1) // 2  # down-proj weight strips stream in two halves

    with tile.TileContext(nc) as tc:
        with (
            tc.tile_pool(name="consts", bufs=1) as cpool,
            tc.tile_pool(name="xpool", bufs=1) as xpool,
            tc.tile_pool(name="wpool", bufs=2) as wpool,
            tc.tile_pool(name="wdpool", bufs=2) as wdpool,
            tc.tile_pool(name="hpool", bufs=it) as hpool,
            tc.tile_pool(name="epool", bufs=2) as epool,
            tc.tile_pool(name="opool", bufs=2) as opool,
            tc.tile_pool(name="psg", bufs=2, space=bass.MemorySpace.PSUM) as psg,
            tc.tile_pool(name="psu", bufs=2, space=bass.MemorySpace.PSUM) as psu,
            tc.tile_pool(name="pso", bufs=2, space=bass.MemorySpace.PSUM) as pso,
        ):
            ag_sb = cpool.tile([P, it], f32, tag="ag")
            au_sb = cpool.tile([P, it], f32, tag="au")
            ad_sb = cpool.tile([P, ht], f32, tag="ad")
            nc.sync.dma_start(ag_sb[:], ags[:])
            nc.sync.dma_start(au_sb[:], aus[:])
            nc.sync.dma_start(ad_sb[:], ads[:])

            def body():
                _build_body(
                    nc, tc, nblk, kt, it, ht, blk,
                    xp, wgp, wup, wdp, outp,
                    ag_sb, au_sb, ad_sb,
                    xpool, wpool, wdpool, hpool, epool, opool, psg, psu, pso,
                    half, bf16, f32, AF, OP, wdt,
                )

            if reps == 1:
                body()
            else:
                with tc.For_i(0, reps, 1):
                    body()
    nc.compile()
    return nc


def _build_body(
    nc, tc, nblk, kt, it, ht, blk,
    xp, wgp, wup, wdp, outp,
    ag_sb, au_sb, ad_sb,
    xpool, wpool, wdpool, hpool, epool, opool, psg, psu, pso,
    half, bf16, f32, AF, OP, wdt,
):
    for b in range(nblk):
                x_sb = xpool.tile([P, kt, blk], bf16, tag="x")
                nc.sync.dma_start(x_sb[:], xp[b])

                h_tiles = []
                for i in range(it):
                    wg_sb = wpool.tile([P, kt, P], wdt, tag="wg")
                    wu_sb = wpool.tile([P, kt, P], wdt, tag="wu")
                    nc.sync.dma_start(wg_sb[:], wgp[i])
                    nc.sync.dma_start(wu_sb[:], wup[i])
                    g_ps = psg.tile([P, blk], f32, tag="g")
                    u_ps = psu.tile([P, blk], f32, tag="u")
                    # CS=128: plain full-array matmuls. Column tiling
                    # (CS=32/64) was measured slower — the per-weight-change
                    # serialization is the array drain, which strips also pay.
                    for n in range(kt):
                        for c in range(0, P, CS):
                            nc.tensor.matmul(
                                g_ps[c : c + CS, :],
                                wg_sb[:, n, c : c + CS],
                                x_sb[:, n, :],
                                start=(n == 0), stop=(n == kt - 1),
                                tile_position=(0, c),
                            )
                    for n in range(kt):
                        for c in range(0, P, CS):
                            nc.tensor.matmul(
                                u_ps[c : c + CS, :],
                                wu_sb[:, n, c : c + CS],
                                x_sb[:, n, :],
                                start=(n == 0), stop=(n == kt - 1),
                                tile_position=(0, c),
                            )
                    s_sb = epool.tile([P, blk], f32, tag="silu")
                    nc.scalar.activation(
                        s_sb[:], g_ps[:], AF.Silu, scale=ag_sb[:, i : i + 1]
                    )
                    h_sb = hpool.tile([P, blk], bf16, tag="h")
                    nc.vector.scalar_tensor_tensor(
                        h_sb[:], u_ps[:], au_sb[:, i : i + 1], s_sb[:],
                        OP.mult, OP.mult,
                    )
                    h_tiles.append(h_sb)

                for o in range(ht):
                    o_ps = pso.tile([P, blk], f32, tag="o")
                    for n0 in range(0, it, half):
                        cnt = min(half, it - n0)
                        wd_sb = wdpool.tile([P, half, P], wdt, tag="wd")
                        nc.sync.dma_start(
                            wd_sb[:, :cnt, :], wdp[o, :, n0 : n0 + cnt, :]
                        )
                        for j in range(cnt):
                            n = n0 + j
                            for c in range(0, P, CS):
                                nc.tensor.matmul(
                                    o_ps[c : c + CS, :],
                                    wd_sb[:, j, c : c + CS],
                                    h_tiles[n][:],
                                    start=(n == 0), stop=(n == it - 1),
                                    tile_position=(0, c),
                                )
                    o_sb = opool.tile([P, blk], f32, tag="osb")
                    nc.vector.tensor_scalar_mul(o_sb[:], o_ps[:], ad_sb[:, o : o + 1])
                    nc.sync.dma_start(outp[b, o], o_sb[:])


def _pack_weight(w, out_tiles, in_tiles):
    # w: [out, in] fp32 -> [out_tile, p_in, n_in, out_col] where
    # packed[i, p, n, ii] = w[i*128+ii, n*128+p]
    o, i = w.shape
    dt = ml_dtypes.float8_e4m3 if WDT == "f8" else ml_dtypes.bfloat16
    return np.ascontiguousarray(
        w.reshape(out_tiles, P, in_tiles, P).transpose(0, 3, 2, 1)
    ).astype(dt)


def _pack_scale(a, tiles):
    # a: [dim] fp32 -> [P, tiles] with packed[p, i] = a[i*128+p]
    return np.ascontiguousarray(a.reshape(tiles, P).T).astype(np.float32)


def prep_inputs(x, Wg, Wu, Wd, ag, au, ad, n_cores=NCORES, blk=BLK):
    """Host-side shard + layout prep. Returns in_maps for run_bass_kernel_spmd."""
    t = x.shape[0] * x.shape[1]
    hid = x.shape[2]
    inter = Wg.shape[0]
    t_loc = t // n_cores
    nblk = t_loc // blk
    kt = hid // P
    it = inter // P
    ht = hid // P

    wgp = _pack_weight(np.asarray(Wg), it, kt)
    wup = _pack_weight(np.asarray(Wu), it, kt)
    wdp = _pack_weight(np.asarray(Wd), ht, it)
    ags = _pack_scale(np.asarray(ag), it)
    aus = _pack_scale(np.asarray(au), it)
    ads = _pack_scale(np.asarray(ad), ht)

    xf = np.asarray(x).reshape(t, hid)
    in_maps = []
    for c in range(n_cores):
        shard = xf[c * t_loc : (c + 1) * t_loc]
        xp = np.ascontiguousarray(
            shard.reshape(nblk, blk, kt, P).transpose(0, 3, 2, 1)
        ).astype(ml_dtypes.bfloat16)
        in_maps.append(
            {"xp": xp, "wgp": wgp, "wup": wup, "wdp": wdp,
             "ags": ags, "aus": aus, "ads": ads}
        )
    return in_maps


def assemble_output(results, b=B, s=S, hid=HID, n_cores=NCORES):
    # per-core outp: [nblk, ht, P, blk] f32 -> [t_loc, hid]
    shards = []
    for c in range(n_cores):
        r = np.asarray(results[c]["outp"])
        nblk, ht, _, blk = r.shape
        shards.append(
            r.transpose(0, 3, 1, 2).reshape(nblk * blk, ht * P)
        )
    out = np.concatenate(shards, axis=0)
    return out.reshape(b, s, hid).astype(np.float32)


_NC_CACHE = {}

def kernel(x, Wg, Wu, Wd, ag, au, ad):
    t = x.shape[0] * x.shape[1]
    t_loc = t // NCORES
    key = (t, x.shape[2], Wg.shape[0])
    if key not in _NC_CACHE:
        _NC_CACHE[key] = build_nc(t_loc, x.shape[2], Wg.shape[0])
    nc = _NC_CACHE[key]
    in_maps = prep_inputs(x, Wg, Wu, Wd, ag, au, ad)
    res = run_bass_kernel_spmd(nc, in_maps, core_ids=list(range(NCORES)))
    return assemble_output(res.results, b=x.shape[0], s=x.shape[1], hid=x.shape[2])

